# revision 1
# baseline (speedup 1.0000x reference)
"""Distributed multi-head attention for Trainium2 (8 NeuronCores).

Problem: B=4, S=2048, D=1024, 16 heads x 64 dim, fp32 I/O.
  q/k/v = hs @ W{q,k,v}.T ; scores = (q/8) @ k.T per (b,h);
  attn = softmax(scores) @ v ; out = attn @ Wo.T

Sharding (tensor-parallel over heads + all-to-all):
  - Each core owns 2 heads (128 channels of Wq/Wk/Wv rows).
  - Every core receives the full hidden_states; computes qT/kT/vT for its
    2 heads over all B*S rows; attention in transposed (scoresT) layout so
    softmax sums come free via a ones-augmented v (no max subtraction:
    scores ~ N(0,1)); per-q normalization via DVE reciprocal + gpsimd
    partition_broadcast.
  - AllToAll redistributes attn_T (bf16): shard j = this core's 2 heads
    for global row block j. After A2A each core holds all 1024 channels
    for its 1024 rows and applies the full Wo locally -> output row shard.

Compute dtype bf16 (rel err ~4e-3), storage fp32.
"""
import os
import numpy as np

B, S, D = 4, 2048, 1024
NCORE = 8
HD = 64
HPC = 2                      # heads per core
CPC = HPC * HD               # channels per core = 128
ROWS = B * S                 # 8192
RPC = ROWS // NCORE          # rows per core = 1024

_CACHE = {}


def _build():
    import concourse.bass as bass
    import concourse.bacc as bacc
    import concourse.mybir as mybir
    import concourse.tile as tile
    from concourse.masks import make_identity

    F32 = mybir.dt.float32
    BF16 = mybir.dt.bfloat16
    AF = mybir.ActivationFunctionType

    nc = bacc.Bacc("TRN2", target_bir_lowering=False, debug=False,
                   num_devices=NCORE)
    hs = nc.dram_tensor("hidden_states", [B, S, D], F32, kind="ExternalInput")
    wq = nc.dram_tensor("Wq", [CPC, D], F32, kind="ExternalInput")
    wk = nc.dram_tensor("Wk", [CPC, D], F32, kind="ExternalInput")
    wv = nc.dram_tensor("Wv", [CPC, D], F32, kind="ExternalInput")
    wo = nc.dram_tensor("Wo", [D, D], F32, kind="ExternalInput")
    out = nc.dram_tensor("out", [RPC, D], F32, kind="ExternalOutput")
    bounce_in = nc.dram_tensor("bounce_in", [NCORE, CPC, RPC], BF16)
    bounce_out = nc.dram_tensor("bounce_out", [NCORE, CPC, RPC], BF16)

    with tile.TileContext(nc) as tc:
        with (
            tc.tile_pool(name="const", bufs=1) as cpool,
            tc.tile_pool(name="persist", bufs=1) as pp,
            tc.tile_pool(name="hsT", bufs=2) as hsT_pool,
            tc.tile_pool(name="proj", bufs=2) as proj_pool,
            tc.tile_pool(name="sb", bufs=3) as sb,
            tc.tile_pool(name="ex", bufs=3) as expool,
            tc.tile_pool(name="ps_sc", bufs=2, space="PSUM") as ps_sc,
            tc.tile_pool(name="ps_av", bufs=2, space="PSUM") as ps_av,
            tc.tile_pool(name="ps_m", bufs=2, space="PSUM") as ps_m,
        ):
            ident = cpool.tile([128, 128], BF16, tag="ident")
            make_identity(nc, ident)

            # ---- weight prep: wT[p] = per-core W shard, transposed to
            # [k(128-part) x out_ch(128)] chunks stacked along free dim.
            wT = {}
            for pname, w in (("q", wq), ("k", wk), ("v", wv)):
                wt = pp.tile([128, D], BF16, tag=f"wT{pname}")
                for kc in range(8):
                    wf = sb.tile([128, 128], F32, tag="wf")
                    nc.sync.dma_start(wf, w[:, kc * 128:(kc + 1) * 128])
                    wb = sb.tile([128, 128], BF16, tag="wb")
                    nc.gpsimd.tensor_copy(wb, wf)
                    pt = ps_m.tile([128, 128], BF16, tag="m")
                    nc.tensor.transpose(pt, wb, ident)
                    nc.vector.tensor_copy(wt[:, kc * 128:(kc + 1) * 128], pt)
                wT[pname] = wt

            # woT[i]: [d(128-part) x e(1024)] = (Wo[:, i*128:(i+1)*128]).T
            woT = []
            for i in range(8):
                wt = pp.tile([128, D], BF16, tag=f"woT{i}")
                for j in range(8):
                    wf = sb.tile([128, 128], F32, tag="wf")
                    nc.sync.dma_start(
                        wf, wo[j * 128:(j + 1) * 128, i * 128:(i + 1) * 128])
                    wb = sb.tile([128, 128], BF16, tag="wb")
                    nc.gpsimd.tensor_copy(wb, wf)
                    pt = ps_m.tile([128, 128], BF16, tag="m")
                    nc.tensor.transpose(pt, wb, ident)
                    nc.vector.tensor_copy(wt[:, j * 128:(j + 1) * 128], pt)
                woT.append(wt)

            # ---- main loop over batches
            for b in range(B):
                # hsT: [k-in-chunk(128-part), kchunk(8), row(2048)] bf16
                hsT = hsT_pool.tile([128, 8, S], BF16, tag="hsT")
                for rt in range(S // 128):
                    hf = sb.tile([128, D], F32, tag="hf")
                    nc.sync.dma_start(hf, hs[b, rt * 128:(rt + 1) * 128, :])
                    hb = sb.tile([128, D], BF16, tag="hb")
                    nc.gpsimd.tensor_copy(hb, hf)
                    for kc in range(8):
                        pt = ps_m.tile([128, 128], BF16, tag="m")
                        nc.tensor.transpose(
                            pt, hb[:, kc * 128:(kc + 1) * 128], ident)
                        nc.vector.tensor_copy(
                            hsT[:, kc, rt * 128:(rt + 1) * 128], pt)

                # projections -> qT/kT/vT [ch(128-part), row(2048)] bf16
                qT = proj_pool.tile([128, S], BF16, tag="qT")
                kTt = proj_pool.tile([128, S], BF16, tag="kT")
                vTt = proj_pool.tile([128, S], BF16, tag="vT")
                for dst, wt, scale in ((qT, wT["q"], 0.125),
                                       (kTt, wT["k"], None),
                                       (vTt, wT["v"], None)):
                    for rb in range(S // 512):
                        pq = ps_m.tile([128, 512], F32, tag="m")
                        for kc in range(8):
                            nc.tensor.matmul(
                                pq,
                                wt[:, kc * 128:(kc + 1) * 128],
                                hsT[:, kc, rb * 512:(rb + 1) * 512],
                                start=(kc == 0), stop=(kc == 7))
                        dslice = dst[:, rb * 512:(rb + 1) * 512]
                        if scale is not None:
                            nc.vector.tensor_scalar_mul(dslice, pq, scale)
                        else:
                            nc.vector.tensor_copy(dslice, pq)

                # v_aug: [krow(128-part), head(2), rowtile(16), 65] bf16
                vaug = proj_pool.tile([128, HPC, S // 128, 65], BF16,
                                      tag="vaug")
                for h in range(HPC):
                    idh = ident[h * 64:(h + 1) * 64, h * 64:(h + 1) * 64]
                    for rt in range(S // 128):
                        pt = ps_m.tile([128, 64], BF16, tag="m")
                        nc.tensor.transpose(
                            pt, vTt[h * 64:(h + 1) * 64,
                                    rt * 128:(rt + 1) * 128], idh)
                        nc.vector.tensor_copy(vaug[:, h, rt, 0:64], pt)
                        nc.vector.memset(vaug[:, h, rt, 64:65], 1.0)

                # attention per (head, q-1024 block)
                for h in range(HPC):
                    hsl = slice(h * 64, (h + 1) * 64)
                    for qp in range(2):
                        q0 = qp * 1024
                        av0 = ps_av.tile([128, 512], F32, tag="av")
                        av1 = ps_av.tile([128, 512], F32, tag="av")
                        for kp in range(S // 128):
                            sc = ps_sc.tile([128, 1024], F32, tag="sc")
                            lk = kTt[hsl, kp * 128:(kp + 1) * 128]
                            nc.tensor.matmul(
                                sc[:, 0:512], lk, qT[hsl, q0:q0 + 512],
                                start=True, stop=True)
                            nc.tensor.matmul(
                                sc[:, 512:1024], lk,
                                qT[hsl, q0 + 512:q0 + 1024],
                                start=True, stop=True)
                            ex = expool.tile([128, 1024], BF16, tag="ex")
                            nc.scalar.activation(ex, sc, AF.Exp)
                            va = vaug[:, h, kp, :]
                            nc.tensor.matmul(av0[0:65, :], va, ex[:, 0:512],
                                             start=(kp == 0), stop=(kp == 15))
                            nc.tensor.matmul(av1[0:65, :], va, ex[:, 512:1024],
                                             start=(kp == 0), stop=(kp == 15))
                        j = b * 2 + qp
                        for half, av in ((0, av0), (1, av1)):
                            recip = sb.tile([1, 512], F32, tag="recip")
                            nc.vector.reciprocal(recip, av[64:65, :])
                            bc = sb.tile([64, 512], F32, tag="bc")
                            nc.gpsimd.partition_broadcast(bc, recip)
                            at = sb.tile([64, 512], BF16, tag="at")
                            nc.vector.tensor_mul(at, av[0:64, :], bc)
                            nc.sync.dma_start(
                                bounce_in[j, hsl,
                                          half * 512:(half + 1) * 512], at)

            # ---- all-to-all: shard j -> core j
            nc.gpsimd.collective_compute(
                "AllToAll", mybir.AluOpType.bypass,
                replica_groups=[list(range(NCORE))],
                ins=[bounce_in[:]], outs=[bounce_out[:]])

            # ---- output projection for this core's 1024 rows
            rcv = []
            for i in range(8):
                rc = pp.tile([128, RPC], BF16, tag=f"rcv{i}")
                nc.sync.dma_start(rc, bounce_out[i])
                rcv.append(rc)
            for m in range(RPC // 128):
                for half in range(2):
                    po = ps_m.tile([128, 512], F32, tag="m")
                    for i in range(8):
                        nc.tensor.matmul(
                            po, rcv[i][:, m * 128:(m + 1) * 128],
                            woT[i][:, half * 512:(half + 1) * 512],
                            start=(i == 0), stop=(i == 7))
                    osb = sb.tile([128, 512], F32, tag="osb")
                    nc.vector.tensor_copy(osb, po)
                    nc.sync.dma_start(
                        out[m * 128:(m + 1) * 128,
                            half * 512:(half + 1) * 512], osb)

    nc.compile()
    return nc


def _get_nc():
    if "nc" not in _CACHE:
        _CACHE["nc"] = _build()
    return _CACHE["nc"]


def kernel(hidden_states, Wq, Wk, Wv, Wo):
    from concourse.bass_utils import run_bass_kernel_spmd

    hidden_states = np.ascontiguousarray(hidden_states, dtype=np.float32)
    Wq = np.ascontiguousarray(Wq, dtype=np.float32)
    Wk = np.ascontiguousarray(Wk, dtype=np.float32)
    Wv = np.ascontiguousarray(Wv, dtype=np.float32)
    Wo = np.ascontiguousarray(Wo, dtype=np.float32)

    nc = _get_nc()
    in_maps = []
    for c in range(NCORE):
        sl = slice(c * CPC, (c + 1) * CPC)
        in_maps.append({
            "hidden_states": hidden_states,
            "Wq": np.ascontiguousarray(Wq[sl]),
            "Wk": np.ascontiguousarray(Wk[sl]),
            "Wv": np.ascontiguousarray(Wv[sl]),
            "Wo": Wo,
        })
    res = run_bass_kernel_spmd(nc, in_maps, list(range(NCORE)))
    full = np.concatenate([res.results[c]["out"] for c in range(NCORE)],
                          axis=0)
    return full.reshape(B, S, D).astype(np.float32)


# revision 8
# speedup vs baseline: 1.1736x; 1.1736x over previous
"""Distributed multi-head attention for Trainium2 (8 NeuronCores).

Problem: B=4, S=2048, D=1024, 16 heads x 64 dim, fp32 I/O.
  q/k/v = hs @ W{q,k,v}.T ; scores = (q/8) @ k.T per (b,h);
  attn = softmax(scores) @ v ; out = attn @ Wo.T

Sharding (tensor-parallel over heads + all-to-all):
  - Each core owns 2 heads (128 channels of Wq/Wk/Wv rows).
  - Every core receives the full hidden_states; computes qT/kT/vT for its
    2 heads over all B*S rows; attention in transposed (scoresT) layout so
    softmax sums come free via a ones-augmented v (no max subtraction:
    scores ~ N(0,1)); per-q normalization via DVE reciprocal + gpsimd
    partition_broadcast.
  - AllToAll redistributes attn_T (bf16): shard j = this core's 2 heads
    for global row block j. After A2A each core holds all 1024 channels
    for its 1024 rows and applies the full Wo locally -> output row shard.

Compute dtype bf16 (rel err ~4e-3), storage fp32.
"""
import os
import numpy as np

B, S, D = 4, 2048, 1024
NCORE = 8
HD = 64
HPC = 2                      # heads per core
CPC = HPC * HD               # channels per core = 128
ROWS = B * S                 # 8192
RPC = ROWS // NCORE          # rows per core = 1024

_CACHE = {}


def _build():
    import concourse.bass as bass
    import concourse.bacc as bacc
    import concourse.mybir as mybir
    import concourse.tile as tile
    from concourse.masks import make_identity

    F32 = mybir.dt.float32
    BF16 = mybir.dt.bfloat16
    AF = mybir.ActivationFunctionType

    nc = bacc.Bacc("TRN2", target_bir_lowering=False, debug=False,
                   num_devices=NCORE)
    hs = nc.dram_tensor("hidden_states", [B, S, D], F32, kind="ExternalInput")
    wq = nc.dram_tensor("Wq", [CPC, D], F32, kind="ExternalInput")
    wk = nc.dram_tensor("Wk", [CPC, D], F32, kind="ExternalInput")
    wv = nc.dram_tensor("Wv", [CPC, D], F32, kind="ExternalInput")
    wo = nc.dram_tensor("Wo", [D, D], F32, kind="ExternalInput")
    out = nc.dram_tensor("out", [RPC, D], F32, kind="ExternalOutput")
    bounce_in = nc.dram_tensor("bounce_in", [NCORE, CPC, RPC], BF16)
    bounce_out = nc.dram_tensor("bounce_out", [NCORE, CPC, RPC], BF16)
    hs16 = nc.dram_tensor("hs16", [B, S, D], BF16)

    with tile.TileContext(nc) as tc:
        with (
            tc.tile_pool(name="const", bufs=1) as cpool,
            tc.tile_pool(name="persist", bufs=1) as pp,
            tc.tile_pool(name="hsT", bufs=2) as hsT_pool,
            tc.tile_pool(name="proj", bufs=2) as proj_pool,
            tc.tile_pool(name="sb", bufs=3) as sb,
            tc.tile_pool(name="ex", bufs=3) as expool,
            tc.tile_pool(name="ps_sc", bufs=2, space="PSUM") as ps_sc,
            tc.tile_pool(name="ps_av", bufs=2, space="PSUM") as ps_av,
            tc.tile_pool(name="ps_m", bufs=2, space="PSUM") as ps_m,
        ):
            ident = cpool.tile([128, 128], BF16, tag="ident")
            make_identity(nc, ident)

            # ---- weight prep: wT[p] = per-core W shard, transposed to
            # [k(128-part) x out_ch(128)] chunks stacked along free dim.
            wT = {}
            for pname, w in (("q", wq), ("k", wk), ("v", wv)):
                wt = pp.tile([128, D], BF16, tag=f"wT{pname}")
                for kc in range(8):
                    wf = sb.tile([128, 128], F32, tag="wf")
                    nc.sync.dma_start(wf, w[:, kc * 128:(kc + 1) * 128])
                    wb = sb.tile([128, 128], BF16, tag="wb")
                    nc.vector.tensor_copy(wb, wf)
                    pt = ps_m.tile([128, 128], BF16, tag="m")
                    nc.tensor.transpose(pt, wb, ident)
                    nc.vector.tensor_copy(wt[:, kc * 128:(kc + 1) * 128], pt)
                wT[pname] = wt

            # woT[i]: [d(128-part) x e(1024)] = (Wo[:, i*128:(i+1)*128]).T
            woT = []
            for i in range(8):
                wt = pp.tile([128, D], BF16, tag=f"woT{i}")
                for j in range(8):
                    wf = sb.tile([128, 128], F32, tag="wf")
                    nc.sync.dma_start(
                        wf, wo[j * 128:(j + 1) * 128, i * 128:(i + 1) * 128])
                    wb = sb.tile([128, 128], BF16, tag="wb")
                    nc.vector.tensor_copy(wb, wf)
                    pt = ps_m.tile([128, 128], BF16, tag="m")
                    nc.tensor.transpose(pt, wb, ident)
                    nc.vector.tensor_copy(wt[:, j * 128:(j + 1) * 128], pt)
                woT.append(wt)

            # ---- main loop over batches
            for b in range(B):
                # cast hs -> bf16 in DRAM (hs16), then DMA-transpose to SBUF
                for rt in range(S // 128):
                    hf = sb.tile([128, D], F32, tag="hf")
                    nc.sync.dma_start(hf, hs[b, rt * 128:(rt + 1) * 128, :])
                    hb = sb.tile([128, D], BF16, tag="hb")
                    nc.vector.tensor_copy(hb, hf)
                    nc.sync.dma_start(
                        hs16[b, rt * 128:(rt + 1) * 128, :], hb)
                # hsT: [k-in-chunk(128-part), kchunk(8), row(2048)] bf16
                hsT = hsT_pool.tile([128, 8, S], BF16, tag="hsT")
                for kc in range(8):
                    nc.sync.dma_start_transpose(
                        hsT[:, kc, :], hs16[b, :, kc * 128:(kc + 1) * 128])

                # projections -> qT/kT/vT [ch(128-part), row(2048)] bf16
                qT = proj_pool.tile([128, S], BF16, tag="qT")
                kTt = proj_pool.tile([128, S], BF16, tag="kT")
                vTt = proj_pool.tile([128, S], BF16, tag="vT")
                for dst, wt, scale in ((qT, wT["q"], 0.125),
                                       (kTt, wT["k"], None),
                                       (vTt, wT["v"], None)):
                    for rb in range(S // 512):
                        pq = ps_m.tile([128, 512], F32, tag="m")
                        for kc in range(8):
                            nc.tensor.matmul(
                                pq,
                                wt[:, kc * 128:(kc + 1) * 128],
                                hsT[:, kc, rb * 512:(rb + 1) * 512],
                                start=(kc == 0), stop=(kc == 7))
                        dslice = dst[:, rb * 512:(rb + 1) * 512]
                        if scale is not None:
                            nc.vector.tensor_scalar_mul(dslice, pq, scale)
                        else:
                            nc.vector.tensor_copy(dslice, pq)

                # v_aug: [krow(128-part), head(2), rowtile(16), 65] bf16
                vaug = proj_pool.tile([128, HPC, S // 128, 65], BF16,
                                      tag="vaug")
                for h in range(HPC):
                    idh = ident[h * 64:(h + 1) * 64, h * 64:(h + 1) * 64]
                    for rt in range(S // 128):
                        pt = ps_m.tile([128, 64], BF16, tag="m")
                        nc.tensor.transpose(
                            pt, vTt[h * 64:(h + 1) * 64,
                                    rt * 128:(rt + 1) * 128], idh)
                        nc.vector.tensor_copy(vaug[:, h, rt, 0:64], pt)
                        nc.vector.memset(vaug[:, h, rt, 64:65], 1.0)

                # attention per (head, q-1024 block)
                for h in range(HPC):
                    hsl = slice(h * 64, (h + 1) * 64)
                    for qp in range(2):
                        q0 = qp * 1024
                        av0 = ps_av.tile([128, 512], F32, tag="av")
                        av1 = ps_av.tile([128, 512], F32, tag="av")
                        for kp in range(S // 128):
                            sc = ps_sc.tile([128, 1024], F32, tag="sc")
                            lk = kTt[hsl, kp * 128:(kp + 1) * 128]
                            nc.tensor.matmul(
                                sc[:, 0:512], lk, qT[hsl, q0:q0 + 512],
                                start=True, stop=True)
                            nc.tensor.matmul(
                                sc[:, 512:1024], lk,
                                qT[hsl, q0 + 512:q0 + 1024],
                                start=True, stop=True)
                            ex = expool.tile([128, 1024], BF16, tag="ex")
                            nc.scalar.activation(ex, sc, AF.Exp)
                            va = vaug[:, h, kp, :]
                            nc.tensor.matmul(av0[0:65, :], va, ex[:, 0:512],
                                             start=(kp == 0), stop=(kp == 15))
                            nc.tensor.matmul(av1[0:65, :], va, ex[:, 512:1024],
                                             start=(kp == 0), stop=(kp == 15))
                        j = b * 2 + qp
                        for half, av in ((0, av0), (1, av1)):
                            ssum = sb.tile([1, 512], F32, tag="ssum")
                            nc.vector.tensor_copy(ssum, av[64:65, :])
                            recip = sb.tile([1, 512], F32, tag="recip")
                            nc.vector.reciprocal_approx_fast(recip, ssum)
                            bc = sb.tile([64, 512], F32, tag="bc")
                            nc.gpsimd.partition_broadcast(bc, recip)
                            at = sb.tile([64, 512], BF16, tag="at")
                            nc.vector.tensor_mul(at, av[0:64, :], bc)
                            nc.sync.dma_start(
                                bounce_in[j, hsl,
                                          half * 512:(half + 1) * 512], at)

            # ---- all-to-all: shard j -> core j
            nc.gpsimd.collective_compute(
                "AllToAll", mybir.AluOpType.bypass,
                replica_groups=[list(range(NCORE))],
                ins=[bounce_in[:]], outs=[bounce_out[:]])

            # ---- output projection for this core's 1024 rows
            rcv = []
            for i in range(8):
                rc = pp.tile([128, RPC], BF16, tag=f"rcv{i}")
                nc.sync.dma_start(rc, bounce_out[i])
                rcv.append(rc)
            for m in range(RPC // 128):
                for half in range(2):
                    po = ps_m.tile([128, 512], F32, tag="m")
                    for i in range(8):
                        nc.tensor.matmul(
                            po, rcv[i][:, m * 128:(m + 1) * 128],
                            woT[i][:, half * 512:(half + 1) * 512],
                            start=(i == 0), stop=(i == 7))
                    osb = sb.tile([128, 512], F32, tag="osb")
                    nc.vector.tensor_copy(osb, po)
                    nc.sync.dma_start(
                        out[m * 128:(m + 1) * 128,
                            half * 512:(half + 1) * 512], osb)

    nc.compile()
    return nc


def _get_nc():
    if "nc" not in _CACHE:
        _CACHE["nc"] = _build()
    return _CACHE["nc"]


def kernel(hidden_states, Wq, Wk, Wv, Wo):
    from concourse.bass_utils import run_bass_kernel_spmd

    hidden_states = np.ascontiguousarray(hidden_states, dtype=np.float32)
    Wq = np.ascontiguousarray(Wq, dtype=np.float32)
    Wk = np.ascontiguousarray(Wk, dtype=np.float32)
    Wv = np.ascontiguousarray(Wv, dtype=np.float32)
    Wo = np.ascontiguousarray(Wo, dtype=np.float32)

    nc = _get_nc()
    in_maps = []
    for c in range(NCORE):
        sl = slice(c * CPC, (c + 1) * CPC)
        in_maps.append({
            "hidden_states": hidden_states,
            "Wq": np.ascontiguousarray(Wq[sl]),
            "Wk": np.ascontiguousarray(Wk[sl]),
            "Wv": np.ascontiguousarray(Wv[sl]),
            "Wo": Wo,
        })
    res = run_bass_kernel_spmd(nc, in_maps, list(range(NCORE)))
    full = np.concatenate([res.results[c]["out"] for c in range(NCORE)],
                          axis=0)
    return full.reshape(B, S, D).astype(np.float32)
